# revision 8
# baseline (speedup 1.0000x reference)
"""Trainium2 Bass kernel: fp8 DoubleRow Euler integrator (8-core data-parallel).

Reference per step: uv = v@U.T; c = (uv*uv)@W.T; x += dt*v (old v);
v += dt*(force - c); x wrapped into [-pi,pi).

Design (per core, batch shard BS=512, transposed on-chip layout [feat, b];
default config: a_mode=fp8, n_streams=2, sq_span=2, uv_bufs=2):

- v state lives in PSUM: psum_v = (v - t*dt*force)/alpha (fp32, one bank
  per d-half). Phase B accumulates -dt*c into it directly, so the only
  per-step DVE state op is the operand cast. The deterministic force ramp
  t*dt*force is never computed on device: the cast re-adds it from a
  host-precomputed bf16 table (v8 = (alpha*pv)*SV + SV*(t+1)*dt*force),
  and the exact fp32 ramp is folded into the x0/vo paths on host.
- Phase A: uv = U@v via fp8 DoubleRow matmuls: one NON-accumulating DR per
  h-tile contracts all of K=256 (both d-halves in the DR pair slots).
  Accumulating matmuls run at half the column rate of start=True matmuls
  (PSUM read-modify-write), so folding the K-chain into the DR pair is
  worth 2.5x on phase-A PE time vs chained bf16.
- Squares: ACT ops of [128, 2*sq_span, CS] over multi-bank PSUM uv tiles,
  sq8 = e4m3(Square(uv * s_act)) written directly as fp8 DoubleRow pair
  tiles for phase B.
- Phase B: 4 accumulating DR matmuls per d-half per step (K_eff=256 each)
  into psum_v.
- Quantization error control (rel_err ~8.6e-3 vs the 2e-2 gate):
  * W8/U8 are scale-dithered: 16 fp8 copies at r_k = 2^(k/16), one per
    step; the dither is cancelled exactly by the per-step ACT scale
    immediate. This turns systematic fp8 weight-quantization error
    (which accumulates linearly over 16 steps) into a random walk.
  * x is NOT integrated from quantized v: xsum (SBUF fp32) accumulates
    alpha*psum_v exactly on DVE each step, eliminating fp8 sampling
    noise from x (the x gate is dominated by +-2pi wrap flips of
    elements near the torus boundary, so x noise matters 6e-3/flip).
  * v0/alpha is injected into psum_v via two bf16 identity matmuls
    (hi/lo bf16 split, error ~2^-16); the final wrap into [-pi,pi) is a
    single +-2pi range reduction (unwrapped x stays in (-2pi, 4pi)).
- n_streams=2 splits the batch into two independent 256-column streams,
  staggering the serial per-step chain (cast -> A -> square -> B -> cast)
  of one stream against engine work of the other; sq_span=2 batches the
  squares of two h-pairs into one ACT op over a 2-bank uv tile.

Per-step engine budget (measured): PE ~3.4us, ACT ~3.7us, DVE ~2.6us;
16-step body ~62-80us vs the 136us f32r baseline.
"""

import contextlib

import numpy as np
import ml_dtypes

import concourse.bacc as bacc
import concourse.mybir as mybir
import concourse.tile as tile
from concourse.bass_utils import run_bass_kernel_spmd

F32 = mybir.dt.float32
BF16 = mybir.dt.bfloat16
FP8 = mybir.dt.float8e4
ALU = mybir.AluOpType
ACTF = mybir.ActivationFunctionType
DR = mybir.MatmulPerfMode.DoubleRow

NPF8 = ml_dtypes.float8_e4m3
NPBF = ml_dtypes.bfloat16

N_CORES = 8
B = 4096
D = 256
H = 1024
P = 128
BS = B // N_CORES          # 512
ND = D // P                # 2 d-halves
NH = H // P                # 8 h-tiles
NPAIR = NH // 2            # 4 h-pairs

DT = np.float32(0.01)
PI = float(np.pi)
TWO_PI = float(2.0 * np.pi)

KW = 16384.0
QS = 4.0
ALPHA = 1.0 / (KW * QS)
SU = 32.0           # fp8 phase-A stationary scale
SV = 16.0           # fp8 phase-A moving scale
GF = 32.0           # force hi-slot stationary scale
N_DITHER = 16

_PROGRAM_CACHE: dict = {}
A_MODE = "fp8"    # "bf16" | "fp8" - selects the phase-A implementation


def _r_k(k):
    return 2.0 ** ((k % N_DITHER) / N_DITHER)


def _q8(x):
    x = np.clip(np.asarray(x, np.float32), -240.0, 240.0)
    return x.astype(NPF8)


def _build(steps: int, loop_reps: int | None = None, variant: str = "full",
           sq_dve_pairs: int = 0, uv_bufs: int = 3, sq_bufs: int = 4,
           a_mode: str = "bf16", cast_cols: int = 0, n_streams: int = 1,
           sq_dve_units: int = 0, sq_span: int = 1):
    """variant: full | mm_only (no squares: B consumes stale sq tiles) |
    sq_only (A+squares, no B/casts).
    a_mode: bf16 (plain bf16 phase A, force folded into casts) |
            fp8 (DoubleRow fp8 phase A, force via fp8 DR pair in phase B).
    cast_cols: if >0, split each cast into column chunks of this width."""
    do_sq = variant in ("full", "sq_only")
    do_b = variant in ("full", "mm_only")
    fp8a = a_mode == "fp8"
    nc = bacc.Bacc(None, target_bir_lowering=False)

    vi0_d = nc.dram_tensor("vi0", [D, BS], BF16, kind="ExternalInput")
    vi1_d = nc.dram_tensor("vi1", [D, BS], BF16, kind="ExternalInput")
    x0_d = nc.dram_tensor("x0pi", [D, BS], F32, kind="ExternalInput")
    ff_d = nc.dram_tensor("ffin", [D, BS], F32, kind="ExternalInput")
    w8_d = nc.dram_tensor("w8d", [N_DITHER * P, NH * D], FP8, kind="ExternalInput")
    idb_d = nc.dram_tensor("idb", [P, P], BF16, kind="ExternalInput")
    tdtf_d = nc.dram_tensor("tdtf", [steps * D, BS], BF16, kind="ExternalInput")
    if fp8a:
        u8_d = nc.dram_tensor("u8d", [N_DITHER * P, 2 * H], FP8, kind="ExternalInput")
        v80_d = nc.dram_tensor("v80", [P, 2 * BS], FP8, kind="ExternalInput")
    else:
        utb_d = nc.dram_tensor("utb", [D, H], BF16, kind="ExternalInput")
        vb0_d = nc.dram_tensor("vb0", [D, BS], BF16, kind="ExternalInput")
    xo_d = nc.dram_tensor("xo", [D, BS], F32, kind="ExternalOutput")
    vo_d = nc.dram_tensor("vo", [D, BS], F32, kind="ExternalOutput")

    nk = min(N_DITHER, steps) if steps > 0 else 1

    with tile.TileContext(nc) as tc:
        with (
            tc.tile_pool(name="state", bufs=1) as state,
            tc.tile_pool(name="sq", bufs=sq_bufs) as sqp,
            tc.tile_pool(name="tmp", bufs=4) as tmp,
            tc.tile_pool(name="psuv", bufs=uv_bufs, space="PSUM") as ps_uv,
            tc.tile_pool(name="psst", bufs=1, space="PSUM") as ps_state,
        ):
            v_bf = [state.tile([P, BS], BF16, name=f"vb{i}") for i in range(ND)]
            w8_s = [state.tile([P, NH, D], FP8, name=f"w8_{k}") for k in range(nk)]
            x0_s = [state.tile([P, BS], F32, name=f"x0_{i}") for i in range(ND)]
            ff_s = [state.tile([P, BS], F32, name=f"ff{i}") for i in range(ND)]
            vi0_s = [state.tile([P, BS], BF16, name=f"vi0_{i}") for i in range(ND)]
            vi1_s = [state.tile([P, BS], BF16, name=f"vi1_{i}") for i in range(ND)]
            idb_s = state.tile([P, P], BF16, name="idb")
            xs_s = [state.tile([P, BS], F32, name=f"xs{i}") for i in range(ND)]
            tdtf_s = [[state.tile([P, BS], BF16, name=f"tf{t}_{i}")
                       for i in range(ND)] for t in range(steps)]
            if fp8a:
                u8_s = [state.tile([P, 2, H], FP8, name=f"u8_{k}") for k in range(nk)]
                v8_s = state.tile([P, 2, BS], FP8, name="v8")
            else:
                ut_s = [state.tile([P, H], BF16, name=f"ut{i}") for i in range(ND)]

            pv = [ps_state.tile([P, BS], F32, name=f"pv{i}") for i in range(ND)]

            def emit_dmas():
                xfers = []
                if fp8a:
                    xfers.append((v8_s[:, 0, :], v80_d[:, 0:BS]))
                    xfers.append((v8_s[:, 1, :], v80_d[:, BS:2 * BS]))
                else:
                    for i in range(ND):
                        xfers.append((v_bf[i][:], vb0_d[i * P:(i + 1) * P, :]))
                xfers.append((idb_s[:], idb_d[:, :]))
                if fp8a:
                    for k in range(nk):
                        xfers.append((u8_s[k][:], u8_d[k * P:(k + 1) * P, :]))
                else:
                    for i in range(ND):
                        xfers.append((ut_s[i][:], utb_d[i * P:(i + 1) * P, :]))
                for i in range(ND):
                    xfers.append((vi0_s[i][:], vi0_d[i * P:(i + 1) * P, :]))
                    xfers.append((vi1_s[i][:], vi1_d[i * P:(i + 1) * P, :]))
                for k in range(nk):
                    xfers.append((w8_s[k][:], w8_d[k * P:(k + 1) * P, :]))
                for t in range(steps):
                    for i in range(ND):
                        xfers.append((tdtf_s[t][i][:],
                                      tdtf_d[(t * ND + i) * P:(t * ND + i + 1) * P, :]))
                for i in range(ND):
                    xfers.append((x0_s[i][:], x0_d[i * P:(i + 1) * P, :]))
                    xfers.append((ff_s[i][:], ff_d[i * P:(i + 1) * P, :]))
                queues = [nc.sync, nc.gpsimd, nc.scalar]
                for qi, (dst, src) in enumerate(xfers):
                    queues[qi % 3].dma_start(dst, src)

            emit_dmas()
            dsq_tiles = None
            if not do_sq:
                dsq_tiles = [state.tile([P, 2, BS], FP8, name=f"dsq{p_}")
                             for p_ in range(NPAIR)]
                for p_ in range(NPAIR):
                    nc.vector.memset(dsq_tiles[p_][:], 0.25)
            for i in range(ND):
                nc.vector.memset(xs_s[i][:], 0.0)
                nc.tensor.matmul(pv[i][:], idb_s[:], vi0_s[i][:],
                                 start=True, stop=False, skip_group_check=True)
                nc.tensor.matmul(pv[i][:], idb_s[:], vi1_s[i][:],
                                 start=False, stop=True, skip_group_check=True)

            CS = BS // n_streams   # columns per stream

            def emit_stream(t, s):
                """One integration step for column stream s (cols c0:c1)."""
                c0, c1 = s * CS, (s + 1) * CS
                k = t % nk
                r = _r_k(t)
                # xsum: exact v_t sample from psum_v (reads pv before B(t,s))
                for i in range(ND):
                    nc.vector.scalar_tensor_tensor(
                        out=xs_s[i][:, c0:c1], in0=pv[i][:, c0:c1],
                        scalar=float(ALPHA),
                        in1=xs_s[i][:, c0:c1], op0=ALU.mult, op1=ALU.add)

                if fp8a:
                    s_act = float(np.sqrt(QS / r) / (SU * SV * r))
                else:
                    s_act = float(np.sqrt(QS / r))
                # sq_span pairs share one uv tile and one ACT square op
                ngrp = NPAIR // sq_span
                sq_grps = []
                for g in range(ngrp):
                    uv_t = ps_uv.tile([P, 2 * sq_span, CS], F32, tag="uv", name="uv")
                    for pp in range(sq_span):
                        p_ = g * sq_span + pp
                        if fp8a:
                            for hh in range(2):
                                ht = 2 * p_ + hh
                                nc.tensor.matmul(
                                    uv_t[:, 2 * pp + hh, :],
                                    u8_s[k][:, :, ht * P:(ht + 1) * P],
                                    v8_s[:, :, c0:c1],
                                    start=True, stop=True, perf_mode=DR,
                                )
                        else:
                            for i in range(ND):
                                for hh in range(2):
                                    ht = 2 * p_ + hh
                                    nc.tensor.matmul(
                                        uv_t[:, 2 * pp + hh, :],
                                        ut_s[i][:, ht * P:(ht + 1) * P],
                                        v_bf[i][:, c0:c1],
                                        start=(i == 0), stop=(i == ND - 1),
                                    )
                    if do_sq:
                        sq_t = sqp.tile([P, 2 * sq_span, CS], FP8, tag="sq", name="sq")
                        on_dve = (s * ngrp + g) >= ngrp * n_streams - sq_dve_units
                        if not on_dve:
                            nc.scalar.activation(sq_t[:], uv_t[:], ACTF.Square,
                                                 scale=s_act)
                        else:
                            uvc = tmp.tile([P, 2 * sq_span, CS], F32, tag="uvc",
                                           name="uvc")
                            nc.vector.tensor_scalar(
                                out=uvc[:], in0=uv_t[:], scalar1=s_act,
                                scalar2=None, op0=ALU.mult)
                            nc.vector.tensor_tensor(
                                out=sq_t[:], in0=uvc[:], in1=uvc[:], op=ALU.mult)
                        sq_grps.append(sq_t)
                    else:
                        sq_grps.append(None)

                if not do_b:
                    return
                # interleaved d-half chains; only the last group's matmuls
                # depend on the final square
                for p_ in range(NPAIR):
                    g, pp = divmod(p_, sq_span)
                    for i in range(ND):
                        if do_sq:
                            rhs = sq_grps[g][:, 2 * pp:2 * pp + 2, :]
                        else:
                            rhs = dsq_tiles[p_][:, :, c0:c1]
                        nc.tensor.matmul(
                            pv[i][:, c0:c1],
                            w8_s[k][:, 2 * p_:2 * p_ + 2, i * P:(i + 1) * P],
                            rhs,
                            start=False, stop=(p_ == NPAIR - 1),
                            perf_mode=DR, skip_group_check=True,
                        )
                # cast for next step's phase A; force ramp re-added here
                # (tdtf_s[t] holds (t+1)*dt*force, pre-scaled by SV for fp8)
                if t + 1 < steps:
                    ccols = cast_cols if cast_cols > 0 else CS
                    for cc0 in range(c0, c1, ccols):
                        cce = min(cc0 + ccols, c1)
                        for i in range(ND):
                            if fp8a:
                                nc.vector.scalar_tensor_tensor(
                                    out=v8_s[:, i, cc0:cce], in0=pv[i][:, cc0:cce],
                                    scalar=float(ALPHA * SV),
                                    in1=tdtf_s[t][i][:, cc0:cce],
                                    op0=ALU.mult, op1=ALU.add)
                            else:
                                nc.vector.scalar_tensor_tensor(
                                    out=v_bf[i][:, cc0:cce], in0=pv[i][:, cc0:cce],
                                    scalar=float(ALPHA),
                                    in1=tdtf_s[t][i][:, cc0:cce],
                                    op0=ALU.mult, op1=ALU.add)

            def emit_step(t):
                for s in range(n_streams):
                    emit_stream(t, s)

            loop_cm = (
                tc.For_i(0, loop_reps, 1,
                         hint_engines=(mybir.EngineType.PE, mybir.EngineType.DVE,
                                       mybir.EngineType.Activation))
                if loop_reps is not None else contextlib.nullcontext()
            )
            with loop_cm:
                for t in range(steps):
                    emit_step(t)

            # final: vo = alpha*pv + steps*dt*force (exact f32 ramp)
            for i in range(ND):
                vo_t = tmp.tile([P, BS], F32, tag="vo", name="vo")
                nc.vector.scalar_tensor_tensor(
                    out=vo_t[:], in0=pv[i][:], scalar=float(ALPHA),
                    in1=ff_s[i][:], op0=ALU.mult, op1=ALU.add)
                xw = tmp.tile([P, BS], F32, tag="xw", name="xw")
                nc.vector.scalar_tensor_tensor(
                    out=xw[:], in0=xs_s[i][:], scalar=float(DT), in1=x0_s[i][:],
                    op0=ALU.mult, op1=ALU.add)
                g = tmp.tile([P, BS], F32, tag="g", name="g")
                nc.vector.tensor_scalar(
                    out=g[:], in0=xw[:], scalar1=TWO_PI, scalar2=None,
                    op0=ALU.is_ge)
                lo = tmp.tile([P, BS], F32, tag="l", name="l")
                nc.vector.tensor_scalar(
                    out=lo[:], in0=xw[:], scalar1=0.0, scalar2=None,
                    op0=ALU.is_lt)
                nc.vector.scalar_tensor_tensor(
                    out=xw[:], in0=g[:], scalar=-TWO_PI, in1=xw[:],
                    op0=ALU.mult, op1=ALU.add)
                nc.vector.scalar_tensor_tensor(
                    out=xw[:], in0=lo[:], scalar=TWO_PI, in1=xw[:],
                    op0=ALU.mult, op1=ALU.add)
                nc.sync.dma_start(xo_d[i * P:(i + 1) * P, :], xw[:])
                nc.scalar.dma_start(vo_d[i * P:(i + 1) * P, :], vo_t[:])

    nc.compile()
    return nc


BEST_CONFIG = {"a_mode": "fp8", "n_streams": 2, "sq_span": 2, "uv_bufs": 2}


def _get_program(steps: int, loop_reps: int | None = None, variant: str = "full",
                 **kw):
    for kk, vv in BEST_CONFIG.items():
        kw.setdefault(kk, vv)
    key = (steps, loop_reps, variant, tuple(sorted(kw.items())))
    if key not in _PROGRAM_CACHE:
        _PROGRAM_CACHE[key] = _build(steps, loop_reps, variant, **kw)
    return _PROGRAM_CACHE[key]


def _make_in_maps(x, v, force, U, W, steps=16, a_mode=None):
    a_mode = a_mode or A_MODE
    fp8a = a_mode == "fp8"
    x = np.asarray(x, np.float32); v = np.asarray(v, np.float32)
    force = np.asarray(force, np.float32)
    U = np.asarray(U, np.float32); W = np.asarray(W, np.float32)

    dtfT = (DT * force.T).astype(np.float32)                # [D,B]
    vT = v.T
    via = (vT.astype(np.float64) / ALPHA)
    vi0 = via.astype(NPBF)
    vi1 = (via - vi0.astype(np.float64)).astype(NPBF)

    wt = (-DT) * W.T.astype(np.float64) * KW                # [H,D]
    nk = min(N_DITHER, steps) if steps > 0 else 1
    w8d = np.zeros((N_DITHER * P, NH * D), NPF8)
    for k in range(nk):
        w8k = _q8(wt * _r_k(k))
        w8d[k * P:(k + 1) * P, :] = w8k.reshape(NH, P, D).transpose(1, 0, 2).reshape(P, NH * D)
    idb = np.eye(P, dtype=np.float32).astype(NPBF)

    base = {"w8d": w8d, "idb": idb}
    # x/v outputs: pv excludes the force ramp in both modes; fold on host
    ramp = float(steps * (steps - 1) / 2)
    x0pi = (x.T + np.float32(PI) + np.float32(ramp * DT) * dtfT).astype(np.float32)
    ffin = (float(steps) * dtfT).astype(np.float32)
    # tdtf[t] holds (t+1)*dt*force (pre-scaled by SV in fp8 mode)
    tsc = SV if fp8a else 1.0
    tdtf = np.zeros((steps * D, B), NPBF)
    for t in range(steps):
        for i in range(ND):
            tdtf[(t * ND + i) * P:(t * ND + i + 1) * P, :] = (
                np.float32(tsc * (t + 1)) * dtfT[i * P:(i + 1) * P, :]).astype(NPBF)
    if fp8a:
        # u8d[k*128+p, half*H + h] = q8(U[h, half*128+p] * SU * r_k)
        utb32 = U.T.astype(np.float64) * SU                  # [D,H]
        u8d = np.zeros((N_DITHER * P, 2 * H), NPF8)
        for k in range(nk):
            q = _q8(utb32 * _r_k(k))                         # [D,H]
            u8d[k * P:(k + 1) * P, 0:H] = q[0:P, :]
            u8d[k * P:(k + 1) * P, H:2 * H] = q[P:2 * P, :]
    else:
        utb = np.ascontiguousarray(U.T).astype(NPBF)         # [D,H]
        vb0 = vT.astype(NPBF)
        base.update({"utb": utb})

    in_maps = []
    for c in range(N_CORES):
        sl = slice(c * BS, (c + 1) * BS)
        m = dict(base)
        m.update({
            "vi0": np.ascontiguousarray(vi0[:, sl]),
            "vi1": np.ascontiguousarray(vi1[:, sl]),
            "x0pi": np.ascontiguousarray(x0pi[:, sl]),
            "ffin": np.ascontiguousarray(ffin[:, sl]),
        })
        m["tdtf"] = np.ascontiguousarray(tdtf[:, sl])
        if fp8a:
            m["u8d"] = u8d
            m["v80"] = np.ascontiguousarray(
                np.concatenate([_q8(vT[0:P, sl] * SV), _q8(vT[P:2 * P, sl] * SV)], axis=1))
        else:
            m["vb0"] = np.ascontiguousarray(vb0[:, sl])
        in_maps.append(m)
    return in_maps


def _run(x, v, force, U, W, steps):
    steps = int(np.asarray(steps).item()) if not isinstance(steps, int) else steps
    if steps == 0:
        return (np.asarray(x, np.float32).copy(),
                np.asarray(v, np.float32).copy())
    nc = _get_program(steps)
    in_maps = _make_in_maps(x, v, force, U, W, steps)
    res = run_bass_kernel_spmd(nc, in_maps, list(range(N_CORES)))
    xo = np.concatenate([res.results[c]["xo"].T for c in range(N_CORES)], axis=0)
    vo = np.concatenate([res.results[c]["vo"].T for c in range(N_CORES)], axis=0)
    xo = (xo - np.float32(PI)).astype(np.float32)
    return xo, vo


def kernel(x, v, force, U, W, steps):
    return _run(x, v, force, U, W, steps)


# revision 10
# speedup vs baseline: 1.0418x; 1.0418x over previous
"""Trainium2 Bass kernel: fp8 DoubleRow Euler integrator (8-core data-parallel).

Reference per step: uv = v@U.T; c = (uv*uv)@W.T; x += dt*v (old v);
v += dt*(force - c); x wrapped into [-pi,pi).

Design (per core, batch shard BS=512, transposed on-chip layout [feat, b];
default config: a_mode=fp8, n_streams=2, sq_span=2, uv_bufs=2):

- v state lives in PSUM: psum_v = (v - t*dt*force)/alpha (fp32, one bank
  per d-half). Phase B accumulates -dt*c into it directly, so the only
  per-step DVE state op is the operand cast. The deterministic force ramp
  t*dt*force is never computed on device: the cast re-adds it from a
  host-precomputed bf16 table (v8 = (alpha*pv)*SV + SV*(t+1)*dt*force),
  and the exact fp32 ramp is folded into the x0/vo paths on host.
- Phase A: uv = U@v via fp8 DoubleRow matmuls: one NON-accumulating DR per
  h-tile contracts all of K=256 (both d-halves in the DR pair slots).
  Accumulating matmuls run at half the column rate of start=True matmuls
  (PSUM read-modify-write), so folding the K-chain into the DR pair is
  worth 2.5x on phase-A PE time vs chained bf16.
- Squares: ACT ops of [128, 2*sq_span, CS] over multi-bank PSUM uv tiles,
  sq8 = e4m3(Square(uv * s_act)) written directly as fp8 DoubleRow pair
  tiles for phase B.
- Phase B: 4 accumulating DR matmuls per d-half per step (K_eff=256 each)
  into psum_v.
- Quantization error control (rel_err ~8.6e-3 vs the 2e-2 gate):
  * W8/U8 are scale-dithered: 16 fp8 copies at r_k = 2^(k/16), one per
    step; the dither is cancelled exactly by the per-step ACT scale
    immediate. This turns systematic fp8 weight-quantization error
    (which accumulates linearly over 16 steps) into a random walk.
  * x is NOT integrated from quantized v: xsum (SBUF fp32) accumulates
    alpha*psum_v exactly on DVE each step, eliminating fp8 sampling
    noise from x (the x gate is dominated by +-2pi wrap flips of
    elements near the torus boundary, so x noise matters 6e-3/flip).
  * v0/alpha is injected into psum_v via two bf16 identity matmuls
    (hi/lo bf16 split, error ~2^-16); the final wrap into [-pi,pi) is a
    single +-2pi range reduction (unwrapped x stays in (-2pi, 4pi)).
- n_streams=2 splits the batch into two independent 256-column streams,
  staggering the serial per-step chain (cast -> A -> square -> B -> cast)
  of one stream against engine work of the other; sq_span=2 batches the
  squares of two h-pairs into one ACT op over a 2-bank uv tile.

Per-step engine budget (measured): PE ~3.4us, ACT ~3.7us, DVE ~2.6us;
16-step body ~62-80us vs the 136us f32r baseline.
"""

import contextlib

import numpy as np
import ml_dtypes

import concourse.bacc as bacc
import concourse.mybir as mybir
import concourse.tile as tile
from concourse.bass_utils import run_bass_kernel_spmd

F32 = mybir.dt.float32
BF16 = mybir.dt.bfloat16
FP8 = mybir.dt.float8e4
ALU = mybir.AluOpType
ACTF = mybir.ActivationFunctionType
DR = mybir.MatmulPerfMode.DoubleRow

NPF8 = ml_dtypes.float8_e4m3
NPBF = ml_dtypes.bfloat16

N_CORES = 8
B = 4096
D = 256
H = 1024
P = 128
BS = B // N_CORES          # 512
ND = D // P                # 2 d-halves
NH = H // P                # 8 h-tiles
NPAIR = NH // 2            # 4 h-pairs

DT = np.float32(0.01)
PI = float(np.pi)
TWO_PI = float(2.0 * np.pi)

KW = 16384.0
QS = 4.0
ALPHA = 1.0 / (KW * QS)
SU = 32.0           # fp8 phase-A stationary scale
SV = 16.0           # fp8 phase-A moving scale
GF = 32.0           # force hi-slot stationary scale
N_DITHER = 16

_PROGRAM_CACHE: dict = {}
A_MODE = "fp8"    # "bf16" | "fp8" - selects the phase-A implementation


def _r_k(k):
    return 2.0 ** ((k % N_DITHER) / N_DITHER)


def _q8(x):
    x = np.clip(np.asarray(x, np.float32), -240.0, 240.0)
    return x.astype(NPF8)


def _build(steps: int, loop_reps: int | None = None, variant: str = "full",
           sq_dve_pairs: int = 0, uv_bufs: int = 3, sq_bufs: int = 4,
           a_mode: str = "bf16", cast_cols: int = 0, n_streams: int = 1,
           sq_dve_units: int = 0, sq_span: int = 1):
    """variant: full | mm_only (no squares: B consumes stale sq tiles) |
    sq_only (A+squares, no B/casts).
    a_mode: bf16 (plain bf16 phase A, force folded into casts) |
            fp8 (DoubleRow fp8 phase A, force via fp8 DR pair in phase B).
    cast_cols: if >0, split each cast into column chunks of this width."""
    do_sq = variant in ("full", "sq_only")
    do_b = variant in ("full", "mm_only")
    fp8a = a_mode == "fp8"
    nc = bacc.Bacc(None, target_bir_lowering=False)

    vi0_d = nc.dram_tensor("vi0", [D, BS], BF16, kind="ExternalInput")
    vi1_d = nc.dram_tensor("vi1", [D, BS], BF16, kind="ExternalInput")
    x0_d = nc.dram_tensor("x0pi", [D, BS], F32, kind="ExternalInput")
    ff_d = nc.dram_tensor("ffin", [D, BS], F32, kind="ExternalInput")
    w8_d = nc.dram_tensor("w8d", [N_DITHER * P, NH * D], FP8, kind="ExternalInput")
    idb_d = nc.dram_tensor("idb", [P, P], BF16, kind="ExternalInput")
    tdtf_d = nc.dram_tensor("tdtf", [steps * D, BS], BF16, kind="ExternalInput")
    if fp8a:
        u8_d = nc.dram_tensor("u8d", [N_DITHER * P, 2 * H], FP8, kind="ExternalInput")
        v80_d = nc.dram_tensor("v80", [P, 2 * BS], FP8, kind="ExternalInput")
    else:
        utb_d = nc.dram_tensor("utb", [D, H], BF16, kind="ExternalInput")
        vb0_d = nc.dram_tensor("vb0", [D, BS], BF16, kind="ExternalInput")
    xo_d = nc.dram_tensor("xo", [D, BS], F32, kind="ExternalOutput")
    vo_d = nc.dram_tensor("vo", [D, BS], F32, kind="ExternalOutput")

    nk = min(N_DITHER, steps) if steps > 0 else 1

    with tile.TileContext(nc) as tc:
        with (
            tc.tile_pool(name="state", bufs=1) as state,
            tc.tile_pool(name="sq", bufs=sq_bufs) as sqp,
            tc.tile_pool(name="tmp", bufs=4) as tmp,
            tc.tile_pool(name="psuv", bufs=uv_bufs, space="PSUM") as ps_uv,
            tc.tile_pool(name="psst", bufs=1, space="PSUM") as ps_state,
        ):
            v_bf = [state.tile([P, BS], BF16, name=f"vb{i}") for i in range(ND)]
            w8_s = [state.tile([P, NH, D], FP8, name=f"w8_{k}") for k in range(nk)]
            x0_s = [state.tile([P, BS], F32, name=f"x0_{i}") for i in range(ND)]
            ff_s = [state.tile([P, BS], F32, name=f"ff{i}") for i in range(ND)]
            vi0_s = [state.tile([P, BS], BF16, name=f"vi0_{i}") for i in range(ND)]
            vi1_s = [state.tile([P, BS], BF16, name=f"vi1_{i}") for i in range(ND)]
            idb_s = state.tile([P, P], BF16, name="idb")
            xs_s = [state.tile([P, BS], F32, name=f"xs{i}") for i in range(ND)]
            tdtf_s = [[state.tile([P, BS], BF16, name=f"tf{t}_{i}")
                       for i in range(ND)] for t in range(steps)]
            if fp8a:
                u8_s = [state.tile([P, 2, H], FP8, name=f"u8_{k}") for k in range(nk)]
                v8_s = state.tile([P, 2, BS], FP8, name="v8")
            else:
                ut_s = [state.tile([P, H], BF16, name=f"ut{i}") for i in range(ND)]

            pv = [ps_state.tile([P, BS], F32, name=f"pv{i}") for i in range(ND)]

            def emit_dmas():
                xfers = []
                if fp8a:
                    xfers.append((v8_s[:, 0, :], v80_d[:, 0:BS]))
                    xfers.append((v8_s[:, 1, :], v80_d[:, BS:2 * BS]))
                else:
                    for i in range(ND):
                        xfers.append((v_bf[i][:], vb0_d[i * P:(i + 1) * P, :]))
                xfers.append((idb_s[:], idb_d[:, :]))
                if fp8a:
                    for k in range(nk):
                        xfers.append((u8_s[k][:], u8_d[k * P:(k + 1) * P, :]))
                else:
                    for i in range(ND):
                        xfers.append((ut_s[i][:], utb_d[i * P:(i + 1) * P, :]))
                for i in range(ND):
                    xfers.append((vi0_s[i][:], vi0_d[i * P:(i + 1) * P, :]))
                    xfers.append((vi1_s[i][:], vi1_d[i * P:(i + 1) * P, :]))
                for k in range(nk):
                    xfers.append((w8_s[k][:], w8_d[k * P:(k + 1) * P, :]))
                for t in range(steps):
                    for i in range(ND):
                        xfers.append((tdtf_s[t][i][:],
                                      tdtf_d[(t * ND + i) * P:(t * ND + i + 1) * P, :]))
                for i in range(ND):
                    xfers.append((x0_s[i][:], x0_d[i * P:(i + 1) * P, :]))
                    xfers.append((ff_s[i][:], ff_d[i * P:(i + 1) * P, :]))
                queues = [nc.sync, nc.gpsimd, nc.scalar]
                for qi, (dst, src) in enumerate(xfers):
                    queues[qi % 3].dma_start(dst, src)

            emit_dmas()
            dsq_tiles = None
            if not do_sq:
                dsq_tiles = [state.tile([P, 2, BS], FP8, name=f"dsq{p_}")
                             for p_ in range(NPAIR)]
                for p_ in range(NPAIR):
                    nc.vector.memset(dsq_tiles[p_][:], 0.25)
            for i in range(ND):
                nc.vector.memset(xs_s[i][:], 0.0)
                nc.tensor.matmul(pv[i][:], idb_s[:], vi0_s[i][:],
                                 start=True, stop=False, skip_group_check=True)
                nc.tensor.matmul(pv[i][:], idb_s[:], vi1_s[i][:],
                                 start=False, stop=True, skip_group_check=True)

            CS = BS // n_streams   # columns per stream

            def emit_stream(t, s):
                """One integration step for column stream s (cols c0:c1)."""
                c0, c1 = s * CS, (s + 1) * CS
                k = t % nk
                r = _r_k(t)
                # xsum: exact v_t sample from psum_v (reads pv before B(t,s))
                for i in range(ND):
                    nc.vector.scalar_tensor_tensor(
                        out=xs_s[i][:, c0:c1], in0=pv[i][:, c0:c1],
                        scalar=float(ALPHA),
                        in1=xs_s[i][:, c0:c1], op0=ALU.mult, op1=ALU.add)

                if fp8a:
                    s_act = float(np.sqrt(QS / r) / (SU * SV * r))
                else:
                    s_act = float(np.sqrt(QS / r))
                # sq_span pairs share one uv tile and one ACT square op
                ngrp = NPAIR // sq_span
                sq_grps = []
                for g in range(ngrp):
                    uv_t = ps_uv.tile([P, 2 * sq_span, CS], F32, tag="uv", name="uv")
                    for pp in range(sq_span):
                        p_ = g * sq_span + pp
                        if fp8a:
                            for hh in range(2):
                                ht = 2 * p_ + hh
                                nc.tensor.matmul(
                                    uv_t[:, 2 * pp + hh, :],
                                    u8_s[k][:, :, ht * P:(ht + 1) * P],
                                    v8_s[:, :, c0:c1],
                                    start=True, stop=True, perf_mode=DR,
                                )
                        else:
                            for i in range(ND):
                                for hh in range(2):
                                    ht = 2 * p_ + hh
                                    nc.tensor.matmul(
                                        uv_t[:, 2 * pp + hh, :],
                                        ut_s[i][:, ht * P:(ht + 1) * P],
                                        v_bf[i][:, c0:c1],
                                        start=(i == 0), stop=(i == ND - 1),
                                    )
                    if do_sq:
                        sq_t = sqp.tile([P, 2 * sq_span, CS], FP8, tag="sq", name="sq")
                        on_dve = (s * ngrp + g) >= ngrp * n_streams - sq_dve_units
                        if not on_dve:
                            nc.scalar.activation(sq_t[:], uv_t[:], ACTF.Square,
                                                 scale=s_act)
                        else:
                            uvc = tmp.tile([P, 2 * sq_span, CS], F32, tag="uvc",
                                           name="uvc")
                            nc.vector.tensor_scalar(
                                out=uvc[:], in0=uv_t[:], scalar1=s_act,
                                scalar2=None, op0=ALU.mult)
                            nc.vector.tensor_tensor(
                                out=sq_t[:], in0=uvc[:], in1=uvc[:], op=ALU.mult)
                        sq_grps.append(sq_t)
                    else:
                        sq_grps.append(None)

                if not do_b:
                    return
                # interleaved d-half chains; only the last group's matmuls
                # depend on the final square
                for p_ in range(NPAIR):
                    g, pp = divmod(p_, sq_span)
                    for i in range(ND):
                        if do_sq:
                            rhs = sq_grps[g][:, 2 * pp:2 * pp + 2, :]
                        else:
                            rhs = dsq_tiles[p_][:, :, c0:c1]
                        nc.tensor.matmul(
                            pv[i][:, c0:c1],
                            w8_s[k][:, 2 * p_:2 * p_ + 2, i * P:(i + 1) * P],
                            rhs,
                            start=False, stop=(p_ == NPAIR - 1),
                            perf_mode=DR, skip_group_check=True,
                        )
                # cast for next step's phase A; force ramp re-added here
                # (tdtf_s[t] holds (t+1)*dt*force, pre-scaled by SV for fp8)
                if t + 1 < steps:
                    ccols = cast_cols if cast_cols > 0 else CS
                    for cc0 in range(c0, c1, ccols):
                        cce = min(cc0 + ccols, c1)
                        for i in range(ND):
                            if fp8a:
                                nc.vector.scalar_tensor_tensor(
                                    out=v8_s[:, i, cc0:cce], in0=pv[i][:, cc0:cce],
                                    scalar=float(ALPHA * SV),
                                    in1=tdtf_s[t][i][:, cc0:cce],
                                    op0=ALU.mult, op1=ALU.add)
                            else:
                                nc.vector.scalar_tensor_tensor(
                                    out=v_bf[i][:, cc0:cce], in0=pv[i][:, cc0:cce],
                                    scalar=float(ALPHA),
                                    in1=tdtf_s[t][i][:, cc0:cce],
                                    op0=ALU.mult, op1=ALU.add)

            def emit_step(t):
                for s in range(n_streams):
                    emit_stream(t, s)

            loop_cm = (
                tc.For_i(0, loop_reps, 1,
                         hint_engines=(mybir.EngineType.PE, mybir.EngineType.DVE,
                                       mybir.EngineType.Activation))
                if loop_reps is not None else contextlib.nullcontext()
            )
            with loop_cm:
                for t in range(steps):
                    emit_step(t)

            # final: vo = alpha*pv + steps*dt*force (exact f32 ramp)
            for i in range(ND):
                vo_t = tmp.tile([P, BS], F32, tag="vo", name="vo")
                nc.vector.scalar_tensor_tensor(
                    out=vo_t[:], in0=pv[i][:], scalar=float(ALPHA),
                    in1=ff_s[i][:], op0=ALU.mult, op1=ALU.add)
                xw = tmp.tile([P, BS], F32, tag="xw", name="xw")
                nc.vector.scalar_tensor_tensor(
                    out=xw[:], in0=xs_s[i][:], scalar=float(DT), in1=x0_s[i][:],
                    op0=ALU.mult, op1=ALU.add)
                g = tmp.tile([P, BS], F32, tag="g", name="g")
                nc.vector.tensor_scalar(
                    out=g[:], in0=xw[:], scalar1=TWO_PI, scalar2=None,
                    op0=ALU.is_ge)
                lo = tmp.tile([P, BS], F32, tag="l", name="l")
                nc.vector.tensor_scalar(
                    out=lo[:], in0=xw[:], scalar1=0.0, scalar2=None,
                    op0=ALU.is_lt)
                nc.vector.scalar_tensor_tensor(
                    out=xw[:], in0=g[:], scalar=-TWO_PI, in1=xw[:],
                    op0=ALU.mult, op1=ALU.add)
                nc.vector.scalar_tensor_tensor(
                    out=xw[:], in0=lo[:], scalar=TWO_PI, in1=xw[:],
                    op0=ALU.mult, op1=ALU.add)
                nc.sync.dma_start(xo_d[i * P:(i + 1) * P, :], xw[:])
                nc.scalar.dma_start(vo_d[i * P:(i + 1) * P, :], vo_t[:])

    nc.compile()
    return nc


BEST_CONFIG = {"a_mode": "fp8", "n_streams": 2, "sq_span": 2, "uv_bufs": 2}


def _get_program(steps: int, loop_reps: int | None = None, variant: str = "full",
                 **kw):
    for kk, vv in BEST_CONFIG.items():
        kw.setdefault(kk, vv)
    key = (steps, loop_reps, variant, tuple(sorted(kw.items())))
    if key not in _PROGRAM_CACHE:
        _PROGRAM_CACHE[key] = _build(steps, loop_reps, variant, **kw)
    return _PROGRAM_CACHE[key]


def _make_in_maps(x, v, force, U, W, steps=16, a_mode=None):
    a_mode = a_mode or A_MODE
    fp8a = a_mode == "fp8"
    x = np.asarray(x, np.float32); v = np.asarray(v, np.float32)
    force = np.asarray(force, np.float32)
    U = np.asarray(U, np.float32); W = np.asarray(W, np.float32)

    dtfT = (DT * force.T).astype(np.float32)                # [D,B]
    vT = v.T
    via = (vT.astype(np.float64) / ALPHA)
    vi0 = via.astype(NPBF)
    vi1 = (via - vi0.astype(np.float64)).astype(NPBF)

    wt = (-DT) * W.T.astype(np.float64) * KW                # [H,D]
    nk = min(N_DITHER, steps) if steps > 0 else 1
    w8d = np.zeros((N_DITHER * P, NH * D), NPF8)
    for k in range(nk):
        w8k = _q8(wt * _r_k(k))
        w8d[k * P:(k + 1) * P, :] = w8k.reshape(NH, P, D).transpose(1, 0, 2).reshape(P, NH * D)
    idb = np.eye(P, dtype=np.float32).astype(NPBF)

    base = {"w8d": w8d, "idb": idb}
    # x/v outputs: pv excludes the force ramp in both modes; fold on host
    ramp = float(steps * (steps - 1) / 2)
    x0pi = (x.T + np.float32(PI) + np.float32(ramp * DT) * dtfT).astype(np.float32)
    ffin = (float(steps) * dtfT).astype(np.float32)
    # tdtf[t] holds (t+1)*dt*force (pre-scaled by SV in fp8 mode)
    tsc = SV if fp8a else 1.0
    tdtf = np.zeros((steps * D, B), NPBF)
    for t in range(steps):
        for i in range(ND):
            tdtf[(t * ND + i) * P:(t * ND + i + 1) * P, :] = (
                np.float32(tsc * (t + 1)) * dtfT[i * P:(i + 1) * P, :]).astype(NPBF)
    if fp8a:
        # u8d[k*128+p, half*H + h] = q8(U[h, half*128+p] * SU * r_k)
        utb32 = U.T.astype(np.float64) * SU                  # [D,H]
        u8d = np.zeros((N_DITHER * P, 2 * H), NPF8)
        for k in range(nk):
            q = _q8(utb32 * _r_k(k))                         # [D,H]
            u8d[k * P:(k + 1) * P, 0:H] = q[0:P, :]
            u8d[k * P:(k + 1) * P, H:2 * H] = q[P:2 * P, :]
    else:
        utb = np.ascontiguousarray(U.T).astype(NPBF)         # [D,H]
        vb0 = vT.astype(NPBF)
        base.update({"utb": utb})

    in_maps = []
    for c in range(N_CORES):
        sl = slice(c * BS, (c + 1) * BS)
        m = dict(base)
        m.update({
            "vi0": np.ascontiguousarray(vi0[:, sl]),
            "vi1": np.ascontiguousarray(vi1[:, sl]),
            "x0pi": np.ascontiguousarray(x0pi[:, sl]),
            "ffin": np.ascontiguousarray(ffin[:, sl]),
        })
        m["tdtf"] = np.ascontiguousarray(tdtf[:, sl])
        if fp8a:
            m["u8d"] = u8d
            m["v80"] = np.ascontiguousarray(
                np.concatenate([_q8(vT[0:P, sl] * SV), _q8(vT[P:2 * P, sl] * SV)], axis=1))
        else:
            m["vb0"] = np.ascontiguousarray(vb0[:, sl])
        in_maps.append(m)
    return in_maps


def _run(x, v, force, U, W, steps):
    steps = int(np.asarray(steps).item()) if not isinstance(steps, int) else steps
    if steps == 0:
        return (np.asarray(x, np.float32).copy(),
                np.asarray(v, np.float32).copy())
    nc = _get_program(steps)
    in_maps = _make_in_maps(x, v, force, U, W, steps)
    res = run_bass_kernel_spmd(nc, in_maps, list(range(N_CORES)))
    xo = np.concatenate([res.results[c]["xo"].T for c in range(N_CORES)], axis=0)
    vo = np.concatenate([res.results[c]["vo"].T for c in range(N_CORES)], axis=0)
    xo = (xo - np.float32(PI)).astype(np.float32)
    return xo, vo


def kernel(x, v, force, U, W, steps):
    return _run(x, v, force, U, W, steps)


# revision 11
# speedup vs baseline: 1.0666x; 1.0238x over previous
"""Trainium2 Bass kernel: fp8 DoubleRow Euler integrator (8-core data-parallel).

Reference per step: uv = v@U.T; c = (uv*uv)@W.T; x += dt*v (old v);
v += dt*(force - c); x wrapped into [-pi,pi).

Design (per core, batch shard BS=512, transposed on-chip layout [feat, b];
default config: a_mode=fp8, n_streams=2, sq_span=2, uv_bufs=2):

- v state lives in PSUM: psum_v = (v - t*dt*force)/alpha (fp32, one bank
  per d-half). Phase B accumulates -dt*c into it directly, so the only
  per-step DVE state op is the operand cast. The deterministic force ramp
  t*dt*force is never computed on device: the cast re-adds it from a
  host-precomputed bf16 table (v8 = (alpha*pv)*SV + SV*(t+1)*dt*force),
  and the exact fp32 ramp is folded into the x0/vo paths on host.
- Phase A: uv = U@v via fp8 DoubleRow matmuls: one NON-accumulating DR per
  h-tile contracts all of K=256 (both d-halves in the DR pair slots).
  Accumulating matmuls run at half the column rate of start=True matmuls
  (PSUM read-modify-write), so folding the K-chain into the DR pair is
  worth 2.5x on phase-A PE time vs chained bf16.
- Squares: ACT ops of [128, 2*sq_span, CS] over multi-bank PSUM uv tiles,
  sq8 = e4m3(Square(uv * s_act)) written directly as fp8 DoubleRow pair
  tiles for phase B.
- Phase B: 4 accumulating DR matmuls per d-half per step (K_eff=256 each)
  into psum_v.
- Quantization error control (rel_err ~8.6e-3 vs the 2e-2 gate):
  * W8/U8 are scale-dithered: 16 fp8 copies at r_k = 2^(k/16), one per
    step; the dither is cancelled exactly by the per-step ACT scale
    immediate. This turns systematic fp8 weight-quantization error
    (which accumulates linearly over 16 steps) into a random walk.
  * x is NOT integrated from quantized v: xsum (SBUF fp32) accumulates
    alpha*psum_v exactly on DVE each step, eliminating fp8 sampling
    noise from x (the x gate is dominated by +-2pi wrap flips of
    elements near the torus boundary, so x noise matters 6e-3/flip).
  * v0/alpha is injected into psum_v via two bf16 identity matmuls
    (hi/lo bf16 split, error ~2^-16); the final wrap into [-pi,pi) is a
    single +-2pi range reduction (unwrapped x stays in (-2pi, 4pi)).
- n_streams=2 splits the batch into two independent 256-column streams,
  staggering the serial per-step chain (cast -> A -> square -> B -> cast)
  of one stream against engine work of the other; sq_span=2 batches the
  squares of two h-pairs into one ACT op over a 2-bank uv tile.

Per-step engine budget (measured): PE ~3.4us, ACT ~3.7us, DVE ~2.6us;
16-step body ~62-80us vs the 136us f32r baseline.
"""

import contextlib

import numpy as np
import ml_dtypes

import concourse.bacc as bacc
import concourse.mybir as mybir
import concourse.tile as tile
from concourse.bass_utils import run_bass_kernel_spmd

F32 = mybir.dt.float32
BF16 = mybir.dt.bfloat16
FP8 = mybir.dt.float8e4
ALU = mybir.AluOpType
ACTF = mybir.ActivationFunctionType
DR = mybir.MatmulPerfMode.DoubleRow

NPF8 = ml_dtypes.float8_e4m3
NPBF = ml_dtypes.bfloat16

N_CORES = 8
B = 4096
D = 256
H = 1024
P = 128
BS = B // N_CORES          # 512
ND = D // P                # 2 d-halves
NH = H // P                # 8 h-tiles
NPAIR = NH // 2            # 4 h-pairs

DT = np.float32(0.01)
PI = float(np.pi)
TWO_PI = float(2.0 * np.pi)

KW = 16384.0
QS = 4.0
ALPHA = 1.0 / (KW * QS)
SU = 32.0           # fp8 phase-A stationary scale
SV = 16.0           # fp8 phase-A moving scale
GF = 32.0           # force hi-slot stationary scale
N_DITHER = 16

_PROGRAM_CACHE: dict = {}
A_MODE = "fp8"    # "bf16" | "fp8" - selects the phase-A implementation


def _r_k(k):
    return 2.0 ** ((k % N_DITHER) / N_DITHER)


def _q8(x):
    x = np.clip(np.asarray(x, np.float32), -240.0, 240.0)
    return x.astype(NPF8)


def _build(steps: int, loop_reps: int | None = None, variant: str = "full",
           sq_dve_pairs: int = 0, uv_bufs: int = 3, sq_bufs: int = 4,
           a_mode: str = "bf16", cast_cols: int = 0, n_streams: int = 1,
           sq_dve_units: int = 0, sq_span: int = 1, sq_sliver: int = 0):
    """variant: full | mm_only (no squares: B consumes stale sq tiles) |
    sq_only (A+squares, no B/casts).
    a_mode: bf16 (plain bf16 phase A, force folded into casts) |
            fp8 (DoubleRow fp8 phase A, force via fp8 DR pair in phase B).
    cast_cols: if >0, split each cast into column chunks of this width."""
    do_sq = variant in ("full", "sq_only")
    do_b = variant in ("full", "mm_only")
    fp8a = a_mode == "fp8"
    nc = bacc.Bacc(None, target_bir_lowering=False)

    vi0_d = nc.dram_tensor("vi0", [D, BS], BF16, kind="ExternalInput")
    vi1_d = nc.dram_tensor("vi1", [D, BS], BF16, kind="ExternalInput")
    x0_d = nc.dram_tensor("x0pi", [D, BS], F32, kind="ExternalInput")
    ff_d = nc.dram_tensor("ffin", [D, BS], F32, kind="ExternalInput")
    w8_d = nc.dram_tensor("w8d", [N_DITHER * P, NH * D], FP8, kind="ExternalInput")
    idb_d = nc.dram_tensor("idb", [P, P], BF16, kind="ExternalInput")
    tdtf_d = nc.dram_tensor("tdtf", [steps * D, BS], BF16, kind="ExternalInput")
    if fp8a:
        u8_d = nc.dram_tensor("u8d", [N_DITHER * P, 2 * H], FP8, kind="ExternalInput")
        v80_d = nc.dram_tensor("v80", [P, 2 * BS], FP8, kind="ExternalInput")
    else:
        utb_d = nc.dram_tensor("utb", [D, H], BF16, kind="ExternalInput")
        vb0_d = nc.dram_tensor("vb0", [D, BS], BF16, kind="ExternalInput")
    xo_d = nc.dram_tensor("xo", [D, BS], F32, kind="ExternalOutput")
    vo_d = nc.dram_tensor("vo", [D, BS], F32, kind="ExternalOutput")

    nk = min(N_DITHER, steps) if steps > 0 else 1

    with tile.TileContext(nc) as tc:
        with (
            tc.tile_pool(name="state", bufs=1) as state,
            tc.tile_pool(name="sq", bufs=sq_bufs) as sqp,
            tc.tile_pool(name="tmp", bufs=4) as tmp,
            tc.tile_pool(name="psuv", bufs=uv_bufs, space="PSUM") as ps_uv,
            tc.tile_pool(name="psst", bufs=1, space="PSUM") as ps_state,
        ):
            v_bf = [state.tile([P, BS], BF16, name=f"vb{i}") for i in range(ND)]
            w8_s = [state.tile([P, NH, D], FP8, name=f"w8_{k}") for k in range(nk)]
            x0_s = [state.tile([P, BS], F32, name=f"x0_{i}") for i in range(ND)]
            ff_s = [state.tile([P, BS], F32, name=f"ff{i}") for i in range(ND)]
            vi0_s = [state.tile([P, BS], BF16, name=f"vi0_{i}") for i in range(ND)]
            vi1_s = [state.tile([P, BS], BF16, name=f"vi1_{i}") for i in range(ND)]
            idb_s = state.tile([P, P], BF16, name="idb")
            xs_s = [state.tile([P, BS], F32, name=f"xs{i}") for i in range(ND)]
            tdtf_s = [[state.tile([P, BS], BF16, name=f"tf{t}_{i}")
                       for i in range(ND)] for t in range(steps)]
            if fp8a:
                u8_s = [state.tile([P, 2, H], FP8, name=f"u8_{k}") for k in range(nk)]
                v8_s = state.tile([P, 2, BS], FP8, name="v8")
            else:
                ut_s = [state.tile([P, H], BF16, name=f"ut{i}") for i in range(ND)]

            pv = [ps_state.tile([P, BS], F32, name=f"pv{i}") for i in range(ND)]

            def emit_dmas():
                xfers = []
                if fp8a:
                    xfers.append((v8_s[:, 0, :], v80_d[:, 0:BS]))
                    xfers.append((v8_s[:, 1, :], v80_d[:, BS:2 * BS]))
                else:
                    for i in range(ND):
                        xfers.append((v_bf[i][:], vb0_d[i * P:(i + 1) * P, :]))
                xfers.append((idb_s[:], idb_d[:, :]))
                if fp8a:
                    for k in range(nk):
                        xfers.append((u8_s[k][:], u8_d[k * P:(k + 1) * P, :]))
                else:
                    for i in range(ND):
                        xfers.append((ut_s[i][:], utb_d[i * P:(i + 1) * P, :]))
                for i in range(ND):
                    xfers.append((vi0_s[i][:], vi0_d[i * P:(i + 1) * P, :]))
                    xfers.append((vi1_s[i][:], vi1_d[i * P:(i + 1) * P, :]))
                for k in range(nk):
                    xfers.append((w8_s[k][:], w8_d[k * P:(k + 1) * P, :]))
                for t in range(steps):
                    for i in range(ND):
                        xfers.append((tdtf_s[t][i][:],
                                      tdtf_d[(t * ND + i) * P:(t * ND + i + 1) * P, :]))
                for i in range(ND):
                    xfers.append((x0_s[i][:], x0_d[i * P:(i + 1) * P, :]))
                    xfers.append((ff_s[i][:], ff_d[i * P:(i + 1) * P, :]))
                queues = [nc.sync, nc.gpsimd, nc.scalar]
                for qi, (dst, src) in enumerate(xfers):
                    queues[qi % 3].dma_start(dst, src)

            emit_dmas()
            dsq_tiles = None
            if not do_sq:
                dsq_tiles = [state.tile([P, 2, BS], FP8, name=f"dsq{p_}")
                             for p_ in range(NPAIR)]
                for p_ in range(NPAIR):
                    nc.vector.memset(dsq_tiles[p_][:], 0.25)
            for i in range(ND):
                nc.vector.memset(xs_s[i][:], 0.0)
                nc.tensor.matmul(pv[i][:], idb_s[:], vi0_s[i][:],
                                 start=True, stop=False, skip_group_check=True)
                nc.tensor.matmul(pv[i][:], idb_s[:], vi1_s[i][:],
                                 start=False, stop=True, skip_group_check=True)

            CS = BS // n_streams   # columns per stream

            def emit_stream(t, s):
                """One integration step for column stream s (cols c0:c1)."""
                c0, c1 = s * CS, (s + 1) * CS
                k = t % nk
                r = _r_k(t)
                # xsum: exact v_t sample from psum_v (reads pv before B(t,s))
                for i in range(ND):
                    nc.vector.scalar_tensor_tensor(
                        out=xs_s[i][:, c0:c1], in0=pv[i][:, c0:c1],
                        scalar=float(ALPHA),
                        in1=xs_s[i][:, c0:c1], op0=ALU.mult, op1=ALU.add)

                if fp8a:
                    s_act = float(np.sqrt(QS / r) / (SU * SV * r))
                else:
                    s_act = float(np.sqrt(QS / r))
                # sq_span pairs share one uv tile and one ACT square op
                ngrp = NPAIR // sq_span
                sq_grps = []
                for g in range(ngrp):
                    uv_t = ps_uv.tile([P, 2 * sq_span, CS], F32, tag="uv", name="uv")
                    for pp in range(sq_span):
                        p_ = g * sq_span + pp
                        if fp8a:
                            for hh in range(2):
                                ht = 2 * p_ + hh
                                nc.tensor.matmul(
                                    uv_t[:, 2 * pp + hh, :],
                                    u8_s[k][:, :, ht * P:(ht + 1) * P],
                                    v8_s[:, :, c0:c1],
                                    start=True, stop=True, perf_mode=DR,
                                )
                        else:
                            for i in range(ND):
                                for hh in range(2):
                                    ht = 2 * p_ + hh
                                    nc.tensor.matmul(
                                        uv_t[:, 2 * pp + hh, :],
                                        ut_s[i][:, ht * P:(ht + 1) * P],
                                        v_bf[i][:, c0:c1],
                                        start=(i == 0), stop=(i == ND - 1),
                                    )
                    if do_sq:
                        sq_t = sqp.tile([P, 2 * sq_span, CS], FP8, tag="sq", name="sq")
                        on_dve = (s * ngrp + g) >= ngrp * n_streams - sq_dve_units
                        if not on_dve and sq_sliver > 0 and s == 0 and g == 0:
                            # column sliver of the first group offloaded to DVE
                            # (early in the step: consumed by B p0, far from
                            # the critical cast path; sized to DVE slack)
                            cs_ = CS - sq_sliver
                            nc.scalar.activation(sq_t[:, :, 0:cs_],
                                                 uv_t[:, :, 0:cs_],
                                                 ACTF.Square, scale=s_act)
                            uvs = tmp.tile([P, 2 * sq_span, sq_sliver], F32,
                                           tag="uvs", name="uvs")
                            nc.vector.tensor_scalar(
                                out=uvs[:], in0=uv_t[:, :, cs_:CS],
                                scalar1=s_act, scalar2=None, op0=ALU.mult)
                            nc.vector.tensor_tensor(
                                out=sq_t[:, :, cs_:CS], in0=uvs[:], in1=uvs[:],
                                op=ALU.mult)
                        elif not on_dve:
                            nc.scalar.activation(sq_t[:], uv_t[:], ACTF.Square,
                                                 scale=s_act)
                        else:
                            uvc = tmp.tile([P, 2 * sq_span, CS], F32, tag="uvc",
                                           name="uvc")
                            nc.vector.tensor_scalar(
                                out=uvc[:], in0=uv_t[:], scalar1=s_act,
                                scalar2=None, op0=ALU.mult)
                            nc.vector.tensor_tensor(
                                out=sq_t[:], in0=uvc[:], in1=uvc[:], op=ALU.mult)
                        sq_grps.append(sq_t)
                    else:
                        sq_grps.append(None)

                if not do_b:
                    return
                # interleaved d-half chains; only the last group's matmuls
                # depend on the final square
                for p_ in range(NPAIR):
                    g, pp = divmod(p_, sq_span)
                    for i in range(ND):
                        if do_sq:
                            rhs = sq_grps[g][:, 2 * pp:2 * pp + 2, :]
                        else:
                            rhs = dsq_tiles[p_][:, :, c0:c1]
                        nc.tensor.matmul(
                            pv[i][:, c0:c1],
                            w8_s[k][:, 2 * p_:2 * p_ + 2, i * P:(i + 1) * P],
                            rhs,
                            start=False, stop=(p_ == NPAIR - 1),
                            perf_mode=DR, skip_group_check=True,
                        )
                # cast for next step's phase A; force ramp re-added here
                # (tdtf_s[t] holds (t+1)*dt*force, pre-scaled by SV for fp8)
                if t + 1 < steps:
                    ccols = cast_cols if cast_cols > 0 else CS
                    for cc0 in range(c0, c1, ccols):
                        cce = min(cc0 + ccols, c1)
                        for i in range(ND):
                            if fp8a:
                                nc.vector.scalar_tensor_tensor(
                                    out=v8_s[:, i, cc0:cce], in0=pv[i][:, cc0:cce],
                                    scalar=float(ALPHA * SV),
                                    in1=tdtf_s[t][i][:, cc0:cce],
                                    op0=ALU.mult, op1=ALU.add)
                            else:
                                nc.vector.scalar_tensor_tensor(
                                    out=v_bf[i][:, cc0:cce], in0=pv[i][:, cc0:cce],
                                    scalar=float(ALPHA),
                                    in1=tdtf_s[t][i][:, cc0:cce],
                                    op0=ALU.mult, op1=ALU.add)

            def emit_step(t):
                for s in range(n_streams):
                    emit_stream(t, s)

            loop_cm = (
                tc.For_i(0, loop_reps, 1,
                         hint_engines=(mybir.EngineType.PE, mybir.EngineType.DVE,
                                       mybir.EngineType.Activation))
                if loop_reps is not None else contextlib.nullcontext()
            )
            with loop_cm:
                for t in range(steps):
                    emit_step(t)

            # final: vo = alpha*pv + steps*dt*force (exact f32 ramp)
            for i in range(ND):
                vo_t = tmp.tile([P, BS], F32, tag="vo", name="vo")
                nc.vector.scalar_tensor_tensor(
                    out=vo_t[:], in0=pv[i][:], scalar=float(ALPHA),
                    in1=ff_s[i][:], op0=ALU.mult, op1=ALU.add)
                xw = tmp.tile([P, BS], F32, tag="xw", name="xw")
                nc.vector.scalar_tensor_tensor(
                    out=xw[:], in0=xs_s[i][:], scalar=float(DT), in1=x0_s[i][:],
                    op0=ALU.mult, op1=ALU.add)
                g = tmp.tile([P, BS], F32, tag="g", name="g")
                nc.vector.tensor_scalar(
                    out=g[:], in0=xw[:], scalar1=TWO_PI, scalar2=None,
                    op0=ALU.is_ge)
                lo = tmp.tile([P, BS], F32, tag="l", name="l")
                nc.vector.tensor_scalar(
                    out=lo[:], in0=xw[:], scalar1=0.0, scalar2=None,
                    op0=ALU.is_lt)
                nc.vector.scalar_tensor_tensor(
                    out=xw[:], in0=g[:], scalar=-TWO_PI, in1=xw[:],
                    op0=ALU.mult, op1=ALU.add)
                nc.vector.scalar_tensor_tensor(
                    out=xw[:], in0=lo[:], scalar=TWO_PI, in1=xw[:],
                    op0=ALU.mult, op1=ALU.add)
                nc.sync.dma_start(xo_d[i * P:(i + 1) * P, :], xw[:])
                nc.scalar.dma_start(vo_d[i * P:(i + 1) * P, :], vo_t[:])

    nc.compile()
    return nc


BEST_CONFIG = {"a_mode": "fp8", "n_streams": 2, "sq_span": 2, "uv_bufs": 2}


def _get_program(steps: int, loop_reps: int | None = None, variant: str = "full",
                 **kw):
    for kk, vv in BEST_CONFIG.items():
        kw.setdefault(kk, vv)
    key = (steps, loop_reps, variant, tuple(sorted(kw.items())))
    if key not in _PROGRAM_CACHE:
        _PROGRAM_CACHE[key] = _build(steps, loop_reps, variant, **kw)
    return _PROGRAM_CACHE[key]


def _make_in_maps(x, v, force, U, W, steps=16, a_mode=None):
    a_mode = a_mode or A_MODE
    fp8a = a_mode == "fp8"
    x = np.asarray(x, np.float32); v = np.asarray(v, np.float32)
    force = np.asarray(force, np.float32)
    U = np.asarray(U, np.float32); W = np.asarray(W, np.float32)

    dtfT = (DT * force.T).astype(np.float32)                # [D,B]
    vT = v.T
    via = (vT.astype(np.float64) / ALPHA)
    vi0 = via.astype(NPBF)
    vi1 = (via - vi0.astype(np.float64)).astype(NPBF)

    wt = (-DT) * W.T.astype(np.float64) * KW                # [H,D]
    nk = min(N_DITHER, steps) if steps > 0 else 1
    w8d = np.zeros((N_DITHER * P, NH * D), NPF8)
    for k in range(nk):
        w8k = _q8(wt * _r_k(k))
        w8d[k * P:(k + 1) * P, :] = w8k.reshape(NH, P, D).transpose(1, 0, 2).reshape(P, NH * D)
    idb = np.eye(P, dtype=np.float32).astype(NPBF)

    base = {"w8d": w8d, "idb": idb}
    # x/v outputs: pv excludes the force ramp in both modes; fold on host
    ramp = float(steps * (steps - 1) / 2)
    x0pi = (x.T + np.float32(PI) + np.float32(ramp * DT) * dtfT).astype(np.float32)
    ffin = (float(steps) * dtfT).astype(np.float32)
    # tdtf[t] holds (t+1)*dt*force (pre-scaled by SV in fp8 mode)
    tsc = SV if fp8a else 1.0
    tdtf = np.zeros((steps * D, B), NPBF)
    for t in range(steps):
        for i in range(ND):
            tdtf[(t * ND + i) * P:(t * ND + i + 1) * P, :] = (
                np.float32(tsc * (t + 1)) * dtfT[i * P:(i + 1) * P, :]).astype(NPBF)
    if fp8a:
        # u8d[k*128+p, half*H + h] = q8(U[h, half*128+p] * SU * r_k)
        utb32 = U.T.astype(np.float64) * SU                  # [D,H]
        u8d = np.zeros((N_DITHER * P, 2 * H), NPF8)
        for k in range(nk):
            q = _q8(utb32 * _r_k(k))                         # [D,H]
            u8d[k * P:(k + 1) * P, 0:H] = q[0:P, :]
            u8d[k * P:(k + 1) * P, H:2 * H] = q[P:2 * P, :]
    else:
        utb = np.ascontiguousarray(U.T).astype(NPBF)         # [D,H]
        vb0 = vT.astype(NPBF)
        base.update({"utb": utb})

    in_maps = []
    for c in range(N_CORES):
        sl = slice(c * BS, (c + 1) * BS)
        m = dict(base)
        m.update({
            "vi0": np.ascontiguousarray(vi0[:, sl]),
            "vi1": np.ascontiguousarray(vi1[:, sl]),
            "x0pi": np.ascontiguousarray(x0pi[:, sl]),
            "ffin": np.ascontiguousarray(ffin[:, sl]),
        })
        m["tdtf"] = np.ascontiguousarray(tdtf[:, sl])
        if fp8a:
            m["u8d"] = u8d
            m["v80"] = np.ascontiguousarray(
                np.concatenate([_q8(vT[0:P, sl] * SV), _q8(vT[P:2 * P, sl] * SV)], axis=1))
        else:
            m["vb0"] = np.ascontiguousarray(vb0[:, sl])
        in_maps.append(m)
    return in_maps


def _run(x, v, force, U, W, steps):
    steps = int(np.asarray(steps).item()) if not isinstance(steps, int) else steps
    if steps == 0:
        return (np.asarray(x, np.float32).copy(),
                np.asarray(v, np.float32).copy())
    nc = _get_program(steps)
    in_maps = _make_in_maps(x, v, force, U, W, steps)
    res = run_bass_kernel_spmd(nc, in_maps, list(range(N_CORES)))
    xo = np.concatenate([res.results[c]["xo"].T for c in range(N_CORES)], axis=0)
    vo = np.concatenate([res.results[c]["vo"].T for c in range(N_CORES)], axis=0)
    xo = (xo - np.float32(PI)).astype(np.float32)
    return xo, vo


def kernel(x, v, force, U, W, steps):
    return _run(x, v, force, U, W, steps)
